# revision 47
# baseline (speedup 1.0000x reference)
"""GQA kernel for TRN2, 8-way tensor-parallel by KV head (v2).

Per core i: KV head i, Q heads 4i..4i+3. All matmuls bf16 (full PE rate at any
free size). Cost-model-driven design:
  - Coalesced DMAs: host pre-lays x^T as [128, 16, B*T] so each 512-col chunk
    loads in ONE descriptor-dense DMA (HWDGE hold is ~625ns per DMA).
  - Scores S^T = K Q^T per 128-key block, two heads side-by-side in one
    [128, 2, 512] PSUM duo tile; ONE exp per block over both heads via 3-D AP,
    diagonal blocks trimmed to the causally valid column window.
  - Causality: diagonal-first AV accumulation with subrange matmuls — invalid
    columns are never streamed, so no zero-memset and no wasted PE rows.
  - Denominator via ones-column in V^T (rides along in the AV matmul).
  - V projected directly in [t, d] orientation (x-chunk stationary), no PE
    transposes.
  - Deferred normalization: Y^T drained unnormalized per chunk; recip (DVE),
    partition-broadcast + multiply (Pool) in chunk-wide ops.
  - Out projection per chunk from normalized Y^T; PSUM staged to SBUF bf16
    (DVE/Act alternating) and DMA'd; host sums the 8 partial outputs.
  - Manual interleave: proj/out-proj matmuls woven between attention blocks so
    the PE never idles during the Act-bound exp phase.
"""

import sys

for p in ("/opt/trn_rl_repo", "/root/.axon_site/_ro/trn_rl_repo"):
    if p not in sys.path:
        sys.path.insert(0, p)

import numpy as np
import ml_dtypes
from collections import deque
from contextlib import ExitStack

import concourse.bacc as bacc
import concourse.mybir as mybir
import concourse.tile as tile

F32 = mybir.dt.float32
BF16 = mybir.dt.bfloat16
BF16_NP = ml_dtypes.bfloat16
EXP = mybir.ActivationFunctionType.Exp

D = 64
ROPE_BASE = 10000.0
AC = 512  # t-chunk


def build_nc(C, T, B):
    CT = C // 128          # contraction tiles (16)
    NCH = T // AC          # chunks per batch (4)
    BT = B * T
    KPB = T // 128         # key blocks per batch (16)

    nc = bacc.Bacc("TRN2", target_bir_lowering=False, debug=False)

    xTr = nc.dram_tensor("xTr", [128, CT, BT], BF16, kind="ExternalInput")
    wqr = nc.dram_tensor("wqr", [128, CT, 256], BF16, kind="ExternalInput")
    wkv = nc.dram_tensor("wkv", [128, CT, 128], BF16, kind="ExternalInput")
    idn = nc.dram_tensor("idn", [64, 64], BF16, kind="ExternalInput")
    wor = nc.dram_tensor("wor", [128, 2, C], BF16, kind="ExternalInput")
    rqc = nc.dram_tensor("rqc", [128, T], BF16, kind="ExternalInput")
    rqs = nc.dram_tensor("rqs", [128, T], BF16, kind="ExternalInput")
    rkc = nc.dram_tensor("rkc", [64, T], BF16, kind="ExternalInput")
    rks = nc.dram_tensor("rks", [64, T], BF16, kind="ExternalInput")
    tri2 = nc.dram_tensor("tri2", [128, 2, 128], BF16, kind="ExternalInput")
    id2 = nc.dram_tensor("id2", [128, 128], BF16, kind="ExternalInput")
    out = nc.dram_tensor("out", [BT, C], BF16, kind="ExternalOutput")

    with tile.TileContext(nc) as tc, ExitStack() as ctx:
        # PSUM: pj 2 + ss 4 + yy 2 = 8 banks
        pj = ctx.enter_context(tc.tile_pool(name="pj", bufs=2, space="PSUM"))
        ss = ctx.enter_context(tc.tile_pool(name="ss", bufs=2, space="PSUM"))
        yy = ctx.enter_context(tc.tile_pool(name="yy", bufs=2, space="PSUM"))

        cst = ctx.enter_context(tc.tile_pool(name="cst", bufs=1))
        xcp = ctx.enter_context(tc.tile_pool(name="xcp", bufs=3))
        ktp = ctx.enter_context(tc.tile_pool(name="ktp", bufs=2))
        vpp = ctx.enter_context(tc.tile_pool(name="vpp", bufs=2))
        qsp = ctx.enter_context(tc.tile_pool(name="qsp", bufs=4))
        qtp = ctx.enter_context(tc.tile_pool(name="qtp", bufs=8))
        tmp = ctx.enter_context(tc.tile_pool(name="tmp", bufs=4))
        ppp = ctx.enter_context(tc.tile_pool(name="ppp", bufs=4))
        ytp = ctx.enter_context(tc.tile_pool(name="ytp", bufs=4))
        osp = ctx.enter_context(tc.tile_pool(name="osp", bufs=4))

        # ---- constants ----
        XW = cst.tile([128, CT, 256], BF16, tag="XW")
        KVW = cst.tile([128, CT, 128], BF16, tag="KVW")
        IDN = cst.tile([64, 64], BF16, tag="IDN")
        ID2 = cst.tile([128, 128], BF16, tag="ID2")
        OW = cst.tile([128, 2, C], BF16, tag="OW")
        RQC = cst.tile([128, T], BF16, tag="RQC")
        RQS = cst.tile([128, T], BF16, tag="RQS")
        RKC = cst.tile([64, T], BF16, tag="RKC")
        RKS = cst.tile([64, T], BF16, tag="RKS")
        TRI = cst.tile([128, 2, 128], BF16, tag="TRI")

        PROJ_ROPES = {}
        XC = {}   # (b, ch) -> x chunk tile [128, CT, AC]
        KT = {}   # b -> [64, T]
        VP = {}   # b -> [128, KPB, 65]
        QT = {}   # (ch%2, h) -> [64, AC]
        YN = {}   # b -> [128, KPB, 256] normalized Y in [t, head*d] layout
        YT = {}   # (b, cl) -> [128, T]

        def emit_xdma(b, ch):
            t = xcp.tile([128, CT, AC], BF16, tag="XC", name=f"XC{b}_{ch}")
            nc.sync.dma_start(t[:, :, :], xTr[:, :, b * T + ch * AC:b * T + (ch + 1) * AC])
            XC[(b, ch)] = t

        def fillers_proj(b, ch):
            """Closures projecting chunk (b, ch): K|V packed, Q0, Q1.

            Wk and Wv ride in one [128c, 128] stationary, so K^T and V^T come
            out of a single [128, AC] moving stream (half the PE rows of
            separate K/V passes). V^T is re-oriented to [t, d] with cheap PE
            transposes (64 out-rows each)."""
            xc = XC[(b, ch)]
            tcol = ch * AC
            res = []

            pkv_box = []
            ks_box, vs_box, qs_box = [], [], {}

            def kv_mm(c0):
                if c0 == 0:
                    pkv_box.append(pj.tile([128, AC], F32, tag="pj", name=f"PKV{b}_{ch}"))
                pkv = pkv_box[0]
                for c in range(c0, c0 + 4):
                    nc.tensor.matmul(pkv[:], KVW[:, c, :], xc[:, c, :],
                                     start=(c == 0), stop=(c == CT - 1))

            def kv_copy():
                ks = qsp.tile([64, AC], BF16, tag="KS", name=f"KS{b}_{ch}")
                nc.vector.tensor_copy(ks[:], pkv_box[0][0:64, :])
                ks_box.append(ks)
                vs = qsp.tile([64, AC], BF16, tag="VS", name=f"VS{b}_{ch}")
                nc.vector.tensor_copy(vs[:], pkv_box[0][64:128, :])
                vs_box.append(vs)

            def k_rope():
                ks = ks_box[0]
                kt = KT[b]
                t1 = tmp.tile([64, AC], BF16, tag="kt1")
                t2 = tmp.tile([64, AC], BF16, tag="kt2")
                nc.vector.tensor_mul(t1[:], ks[:], RKC[:, tcol:tcol + AC])
                nc.vector.tensor_mul(t2[0:32, :], ks[32:64, :], RKS[32:64, tcol:tcol + AC])
                nc.vector.tensor_mul(t2[32:64, :], ks[0:32, :], RKS[0:32, tcol:tcol + AC])
                nc.vector.tensor_add(kt[:, tcol:tcol + AC], t1[:], t2[:])

            def v_trans():
                pvt = pj.tile([128, 4, 64], BF16, tag="pj", name=f"PVT{b}_{ch}")
                for tb in range(4):
                    nc.tensor.transpose(pvt[:, tb, :],
                                        vs_box[0][:, tb * 128:(tb + 1) * 128],
                                        IDN[:, :])
                nc.vector.tensor_copy(VP[b][:, ch * 4:(ch + 1) * 4, 0:64],
                                      pvt[:, :, :])

            def q_mm(c0, hp, pq_box):
                if c0 == 0:
                    pq_box.append(pj.tile([128, AC], F32, tag="pj", name=f"PQ{b}_{ch}_{hp}"))
                pq = pq_box[0]
                for c in range(c0, c0 + 4):
                    nc.tensor.matmul(pq[:], XW[:, c, hp * 128:(hp + 1) * 128], xc[:, c, :],
                                     start=(c == 0), stop=(c == CT - 1))

            def q_copy(hp, pq_box):
                qs = qsp.tile([128, AC], BF16, tag="QS", name=f"QS{b}_{ch}_{hp}")
                nc.vector.tensor_copy(qs[:], pq_box[0][:])
                qs_box[hp] = qs

            def q_rope(hp):
                qs = qs_box[hp]
                t1 = tmp.tile([128, AC], BF16, tag="qt1")
                t2 = tmp.tile([128, AC], BF16, tag="qt2")
                nc.vector.tensor_mul(t1[:], qs[:], RQC[:, tcol:tcol + AC])
                for b0 in (0, 64):
                    nc.vector.tensor_mul(t2[b0:b0 + 32, :], qs[b0 + 32:b0 + 64, :],
                                         RQS[b0 + 32:b0 + 64, tcol:tcol + AC])
                    nc.vector.tensor_mul(t2[b0 + 32:b0 + 64, :], qs[b0:b0 + 32, :],
                                         RQS[b0:b0 + 32, tcol:tcol + AC])
                for hl in range(2):
                    h = 2 * hp + hl
                    qt = qtp.tile([64, AC], BF16, tag="QT", name=f"QT{b}_{ch}_{h}")
                    nc.vector.tensor_add(qt[:], t1[hl * 64:hl * 64 + 64, :],
                                         t2[hl * 64:hl * 64 + 64, :])
                    QT[(ch % 2, h)] = qt

            # psum->sbuf copies right behind each chain (frees pj bufs fast);
            # ropes returned separately so chunk+2 pushes can defer them
            # (QT parity), but the chunk+1 push weaves them inline
            for c0 in range(0, CT, 4):
                res.append((875, lambda c0=c0: kv_mm(c0)))
            res.append((15, kv_copy))
            pq_boxes = [[], []]
            for hp in range(2):
                for c0 in range(0, CT, 4):
                    res.append((875, lambda c0=c0, hp=hp: q_mm(c0, hp, pq_boxes[hp])))
                res.append((10, lambda hp=hp: q_copy(hp, pq_boxes[hp])))
                if hp == 0:
                    res.append((160, v_trans))
            ropes = [(150, k_rope), (150, lambda: q_rope(0)),
                     (150, lambda: q_rope(1))]
            return res, ropes

        def weave(ca, ra):
            """Interleave ropes right behind their producing copies:
            k_rope after kv_copy (idx 4), q_rope0 after q_copy0 (idx 9),
            q_rope1 after q_copy1 (idx 16)."""
            return (ca[0:5] + [ra[0]] + ca[5:10] + [ra[1]]
                    + ca[10:16] + [ra[2]] + ca[16:])

        TAIL = [False]

        def fillers_outproj(b, ch):
            """Closures for out projection of chunk (b, ch) (needs YT cols).

            Two co-columns per group share one [128,1024] staging tile and a
            single DMA — halves the serialized HWDGE holds (625ns each)."""
            res = []

            os_box = {}
            did0 = {}

            def po_group(tt, co2):
                trow = b * T + ch * AC + tt * 128
                if co2 == 0:
                    os_box[tt] = osp.tile([128, 2048], BF16, tag="OS", name=f"OS{b}_{ch}_{tt}")
                os_ = os_box[tt]
                for j in range(2):
                    co = 2 * co2 + j
                    po = pj.tile([128, 512], F32, tag="pj", name=f"PO{b}_{ch}_{tt}_{co}")
                    for cl in range(2):
                        nc.tensor.matmul(po[:], YT[(b, cl)][:, ch * 4 * 128 + tt * 128:ch * 4 * 128 + (tt + 1) * 128],
                                         OW[:, cl, co * 512:(co + 1) * 512],
                                         start=(cl == 0), stop=(cl == 1))
                    if j == 0:
                        nc.vector.tensor_copy(os_[:, co2 * 1024:co2 * 1024 + 512], po[:])
                    else:
                        nc.vector.tensor_copy(os_[:, co2 * 1024 + 512:co2 * 1024 + 1024], po[:])
                if co2 == 0:
                    did0[tt] = TAIL[0]
                    if TAIL[0]:
                        nc.sync.dma_start(out[trow:trow + 128, 0:1024], os_[:, 0:1024])
                elif did0[tt]:
                    nc.sync.dma_start(out[trow:trow + 128, 1024:2048], os_[:, 1024:2048])
                else:
                    nc.sync.dma_start(out[trow:trow + 128, :], os_[:])

            for tt in range(4):
                for co2 in range(C // 1024):
                    res.append((880, lambda tt=tt, co2=co2: po_group(tt, co2)))
            return res

        projq = deque()   # (cost_ns, closure) — must drain before next chunk
        sideq = deque()   # (cost, closure, min_epoch) — out-proj, drain lazily
        qcost = [0]       # total cost queued
        pcost = [0]       # proj cost queued
        epoch = [0]       # attention blocks emitted so far
        PROJ_FLOOR = 4000  # keep this much proj work for the chunk boundary

        def push(q, items):
            q.extend(items)
            qcost[0] += sum(it[0] for it in items)
            if q is projq:
                pcost[0] += sum(it[0] for it in items)

        reserve = [0]

        def side_ready():
            return (sideq and qcost[0] > reserve[0]
                    and epoch[0] >= sideq[0][2])

        def head_cost():
            if projq:
                return projq[0][0]
            if side_ready():
                return sideq[0][0]
            return None

        def fill(budget):
            while budget > 0:
                if projq and pcost[0] > PROJ_FLOOR:
                    c, f = projq.popleft()
                    pcost[0] -= c
                elif side_ready():
                    c, f, _ = sideq.popleft()
                elif projq:
                    c, f = projq.popleft()
                    pcost[0] -= c
                else:
                    return
                f()
                qcost[0] -= c
                budget -= c

        def flush_proj():
            while projq:
                c, f = projq.popleft()
                qcost[0] -= c
                pcost[0] -= c
                f()

        def flush_all():
            flush_proj()
            while sideq:
                c, f, _ = sideq.popleft()
                qcost[0] -= c
                f()

        def normalize_flip(b, ch, yj, pr, last=False):
            """Stage Y [t, 4qb, d|den] to SBUF f32; den is per-PARTITION in
            this orientation, so one recip + per-partition-scalar muls —
            no broadcast needed."""
            for j in range(2):
                ys = tmp.tile([128, 4, 65], F32, tag="YS", name=f"YS{b}_{ch}_{pr}_{j}")
                nc.vector.tensor_copy(ys[:, :, :], yj[j][:, :, 0:65])
                rd = tmp.tile([128, 4], F32, tag="RD", name=f"RD{b}_{ch}_{pr}_{j}")
                nc.vector.reciprocal(rd[:, :], ys[:, :, 64])
                h = 2 * pr + j
                for qb in range(4):
                    nc.vector.tensor_scalar_mul(
                        YN[b][:, ch * 4 + qb, h * 64:(h + 1) * 64],
                        ys[:, qb, 0:64], rd[:, qb:qb + 1])

        def fillers_ytrans(b, ch, cl):
            """Transpose normalized Y [t, hd] back to [hd, t] for out-proj."""
            def yt_go():
                pvy = pj.tile([128, 4, 128], BF16, tag="pj", name=f"PYT{b}_{ch}_{cl}")
                for tt in range(4):
                    nc.tensor.transpose(pvy[:, tt, :],
                                        YN[b][:, ch * 4 + tt, cl * 128:(cl + 1) * 128],
                                        ID2[:, :])
                nc.vector.tensor_copy(YT[(b, cl)][:, ch * AC:(ch + 1) * AC],
                                      pvy[:, :, :])
            return [(280, yt_go)]

        def attention_chunk(b, ch, budget, last_chunk):
            """Both head-pair passes of chunk (b, ch) as one pipelined stream.

            AV is q-stationary: each exp'd P block [keys, 128q] loads as the
            (free) stationary and V [keys, 65] streams through, so an entry
            costs 65-row matmuls instead of 512 — Y lands as [q, d|den]."""
            kt = KT[b]
            vp = VP[b]
            kis = [("d", l) for l in range(4)] + [("f", k) for k in range(ch * 4)]
            n = len(kis)
            yps = {}
            pend = deque()

            def emit_av(e):
                p2, f0, pr, idx, ki_ = e
                qb0 = f0 // 128
                for j in range(2):
                    for qb in range(qb0, 4):
                        # ONE accumulation group per head-bank: start zeroes
                        # the whole 2kb zero region, so only the first matmul
                        # into the bank starts and only the last stops.
                        nc.tensor.matmul(yps[pr][j][:, qb, 0:65],
                                         p2[:, j, qb * 128:(qb + 1) * 128],
                                         vp[:, ki_, :],
                                         start=(idx == 0 and qb == 0),
                                         stop=(idx == n - 1 and qb == 3),
                                         skip_group_check=True)
                if idx == n - 1:
                    normalize_flip(b, ch, yps[pr], pr, last_chunk and pr == 1)
                    push(sideq, [(c, f, epoch[0] + 2)
                                 for c, f in fillers_ytrans(b, ch, pr)])

            for pr in range(2):
                for i, (kind, v) in enumerate(kis):
                    ki = ch * 4 + v if kind == "d" else v
                    f0 = v * 128 if kind == "d" else 0
                    if i == 0:
                        yps[pr] = [yy.tile([128, 4, 128], F32, tag="yy",
                                           name=f"Y{b}_{ch}_{pr}_{j}")
                                   for j in range(2)]
                    s2 = ss.tile([128, 2, 512], F32, tag="ss", name=f"S{b}_{ch}_{pr}_{i}")
                    for j in range(2):
                        h = 2 * pr + j
                        nc.tensor.matmul(s2[:, j, f0:512], kt[:, ki * 128:(ki + 1) * 128],
                                         QT[(ch % 2, h)][:, f0:512], start=True, stop=True)
                    p2 = ppp.tile([128, 2, 512], BF16, tag="P2")
                    nc.scalar.activation(p2[:, :, f0:512], s2[:, :, f0:512], EXP)
                    if kind == "d":
                        nc.gpsimd.tensor_mul(p2[:, :, f0:f0 + 128], p2[:, :, f0:f0 + 128], TRI[:, :, :])
                    pend.append((p2, f0, pr, i, ki))
                    epoch[0] += 1
                    if len(pend) > 2:
                        emit_av(pend.popleft())
                    fill(800)
            while pend:
                emit_av(pend.popleft())

        # ---- PE warmup: ramp the p-state while the first DMAs land.
        #      8 matmuls end ~3.2us — exactly when the ramp completes and
        #      the first weights + x rows have arrived. ----
        WRM = cst.tile([128, 512], BF16, tag="WRM")
        nc.vector.memset(WRM[:, :], 0.0)
        pwarm = pj.tile([128, 512], F32, tag="pj", name="PWARM")
        for _ in range(8):
            nc.tensor.matmul(pwarm[:], WRM[:, 0:128], WRM[:], start=True, stop=True)

        # ---- preamble DMAs (need-ordered; x chunk 0 and XW split so each
        #      proj chain can start as soon as its first operands arrive) ----
        x00 = xcp.tile([128, CT, AC], BF16, tag="XC", name="XC0_0")
        XC[(0, 0)] = x00
        nc.sync.dma_start(KVW[:, 0:8, :], wkv[:, 0:8, :])
        nc.sync.dma_start(x00[:, 0:4, :], xTr[:, 0:4, 0:AC])
        nc.sync.dma_start(KVW[:, 8:CT, :], wkv[:, 8:CT, :])
        for q in range(1, 4):
            nc.sync.dma_start(x00[:, 4 * q:4 * (q + 1), :],
                              xTr[:, 4 * q:4 * (q + 1), 0:AC])
        nc.sync.dma_start(IDN[:], idn[:])
        nc.sync.dma_start(RKC[:], rkc[:])
        nc.sync.dma_start(RKS[:], rks[:])
        nc.sync.dma_start(XW[:, 0:4, :], wqr[:, 0:4, :])
        nc.sync.dma_start(XW[:, 4:CT, :], wqr[:, 4:CT, :])
        nc.sync.dma_start(RQC[:], rqc[:])
        nc.sync.dma_start(RQS[:], rqs[:])
        nc.sync.dma_start(TRI[:, :, :], tri2[:, :, :])
        nc.sync.dma_start(ID2[:, :], id2[:, :])
        nc.sync.dma_start(OW[:, :, :], wor[:, :, :])
        emit_xdma(0, 1)

        for b in range(B):
            KT[b] = ktp.tile([64, T], BF16, tag="KT", name=f"KT{b}")
            VP[b] = vpp.tile([128, KPB, 65], BF16, tag="VP", name=f"VP{b}")
            nc.vector.memset(VP[b][:, :, 64:65], 1.0)
            YN[b] = ytp.tile([128, KPB, 256], BF16, tag="YN", name=f"YN{b}",
                             bufs=2)
            for cl in range(2):
                YT[(b, cl)] = ytp.tile([128, T], BF16, tag="YT", name=f"YT{b}_{cl}")

        def warm():
            nc.tensor.matmul(pwarm[:], WRM[:, 0:128], WRM[:], start=True, stop=True)

        ca, ra = fillers_proj(0, 0)
        for _, c in ca + ra:
            c()
        for _ in range(6):
            warm()

        def succ(b, ch, k):
            t = b * NCH + ch + k
            return (t // NCH, t % NCH) if t < B * NCH else None

        pushed = set()
        for b in range(B):
            for ch in range(NCH):
                flush_proj()  # this chunk's proj must be complete
                nxt = succ(b, ch, 2)
                if nxt:
                    emit_xdma(*nxt)
                # queue fillers: chains for chunk+1 (if new) + its ropes,
                # then chains of chunk+2 (QT-parity-safe extra supply)
                nxt = succ(b, ch, 1)
                if nxt:
                    if nxt not in pushed:
                        ca, ra = fillers_proj(*nxt)
                        push(projq, ca)
                        pushed.add(nxt)
                        PROJ_ROPES[nxt] = ra
                    push(projq, PROJ_ROPES.pop(nxt))
                nxt = succ(b, ch, 2)
                if ch >= 2 and nxt and nxt not in pushed:
                    ca, ra = fillers_proj(*nxt)
                    push(projq, ca)
                    pushed.add(nxt)
                    PROJ_ROPES[nxt] = ra
                last = b == B - 1 and ch == NCH - 1
                reserve[0] = 6000 if last else 0
                attention_chunk(b, ch, 800, last)
                # PO of this chunk only becomes eligible a few attention
                # blocks into the NEXT chunk, so normalize has drained.
                push(sideq, [(c, f, epoch[0] + 3)
                             for c, f in fillers_outproj(b, ch)])
        TAIL[0] = True
        flush_all()

    nc.compile()
    return nc


def rope_tables(T, scale):
    inv = 1.0 / (ROPE_BASE ** (np.arange(0, D, 2, dtype=np.float32) / D))
    t = np.arange(T, dtype=np.float32)
    freqs = np.outer(t, inv)
    emb = np.concatenate([freqs, freqs], -1)
    cos = np.cos(emb).T.astype(np.float32) * scale
    sin = np.sin(emb).T.astype(np.float32) * scale
    sinX = np.empty((64, T), np.float32)
    sinX[0:32] = sin[32:64]
    sinX[32:64] = -sin[0:32]
    return np.ascontiguousarray(cos), np.ascontiguousarray(sinX)


def _pk(a, nblk):
    """[nblk*128, F] -> [128, nblk, F] contiguous bf16."""
    n, f = a.shape
    return np.ascontiguousarray(
        a.reshape(nblk, 128, f).transpose(1, 0, 2)).astype(BF16_NP)


def make_inputs(x, Wq, Wk, Wv, Wo):
    B, T, C = x.shape
    CT = C // 128
    xT = np.ascontiguousarray(x.reshape(B * T, C).T)
    qc, qs = rope_tables(T, 1.0 / np.sqrt(D).astype(np.float32))
    kc, ks = rope_tables(T, 1.0)
    tri = np.triu(np.ones((128, 128), np.float32))
    common = {
        "xTr": _pk(xT, CT),
        "rqc": np.concatenate([qc, qc], 0).astype(BF16_NP),
        "rqs": np.concatenate([qs, qs], 0).astype(BF16_NP),
        "rkc": kc.astype(BF16_NP),
        "rks": ks.astype(BF16_NP),
        "tri2": np.ascontiguousarray(
            np.stack([tri, tri], 1)).astype(BF16_NP),
        "idn": np.eye(64, dtype=np.float32).astype(BF16_NP),
        "id2": np.eye(128, dtype=np.float32).astype(BF16_NP),
    }
    in_maps = []
    for i in range(8):
        m = dict(common)
        m["wqr"] = _pk(np.ascontiguousarray(Wq[:, i * 256:(i + 1) * 256]), CT)
        m["wkv"] = _pk(np.ascontiguousarray(np.concatenate(
            [Wk[:, i * 64:(i + 1) * 64], Wv[:, i * 64:(i + 1) * 64]], 1)), CT)
        m["wor"] = _pk(np.ascontiguousarray(Wo[i * 256:(i + 1) * 256, :]), 2)
        in_maps.append(m)
    return in_maps


_NC_CACHE = {}


def _get_nc(C, T, B):
    key = (C, T, B)
    if key not in _NC_CACHE:
        _NC_CACHE[key] = build_nc(C, T, B)
    return _NC_CACHE[key]


def run(x, Wq, Wk, Wv, Wo, trace=False):
    from concourse.bass_utils import run_bass_kernel_spmd

    B, T, C = x.shape
    nc = _get_nc(C, T, B)
    in_maps = make_inputs(x, Wq, Wk, Wv, Wo)
    for attempt in range(3):
        try:
            res = run_bass_kernel_spmd(nc, in_maps, list(range(8)), trace=trace)
        except (ImportError, ModuleNotFoundError):
            res = run_bass_kernel_spmd(nc, in_maps, list(range(8)), trace=False)
        acc = res.results[0]["out"].astype(np.float32)
        for i in range(1, 8):
            acc = acc + res.results[i]["out"].astype(np.float32)
        if np.isfinite(acc).all():
            break
    return acc.reshape(B, T, C), res


def kernel(x, Wq, Wk, Wv, Wo):
    out, _ = run(x, Wq, Wk, Wv, Wo, trace=False)
    return out



# revision 51
# speedup vs baseline: 1.0721x; 1.0721x over previous
"""GQA kernel for TRN2, 8-way tensor-parallel by KV head (v2).

Per core i: KV head i, Q heads 4i..4i+3. All matmuls bf16 (full PE rate at any
free size). Cost-model-driven design:
  - Coalesced DMAs: host pre-lays x^T as [128, 16, B*T] so each 512-col chunk
    loads in ONE descriptor-dense DMA (HWDGE hold is ~625ns per DMA).
  - Scores S^T = K Q^T per 128-key block, two heads side-by-side in one
    [128, 2, 512] PSUM duo tile; ONE exp per block over both heads via 3-D AP,
    diagonal blocks trimmed to the causally valid column window.
  - Causality: diagonal-first AV accumulation with subrange matmuls — invalid
    columns are never streamed, so no zero-memset and no wasted PE rows.
  - Denominator via ones-column in V^T (rides along in the AV matmul).
  - V projected directly in [t, d] orientation (x-chunk stationary), no PE
    transposes.
  - Deferred normalization: Y^T drained unnormalized per chunk; recip (DVE),
    partition-broadcast + multiply (Pool) in chunk-wide ops.
  - Out projection per chunk from normalized Y^T; PSUM staged to SBUF bf16
    (DVE/Act alternating) and DMA'd; host sums the 8 partial outputs.
  - Manual interleave: proj/out-proj matmuls woven between attention blocks so
    the PE never idles during the Act-bound exp phase.
"""

import sys

for p in ("/opt/trn_rl_repo", "/root/.axon_site/_ro/trn_rl_repo"):
    if p not in sys.path:
        sys.path.insert(0, p)

import numpy as np
import ml_dtypes
from collections import deque
from contextlib import ExitStack

import concourse.bacc as bacc
import concourse.mybir as mybir
import concourse.tile as tile

F32 = mybir.dt.float32
BF16 = mybir.dt.bfloat16
BF16_NP = ml_dtypes.bfloat16
EXP = mybir.ActivationFunctionType.Exp

D = 64
ROPE_BASE = 10000.0
AC = 512  # t-chunk

# scheduler knobs (swept offline; see sweep.py)
PARAMS = dict(FILL=800, PEND=9, FLOOR=0, RESERVE=9000, EGATE=5, YGATE=2,
              WPRE=8, WPOST=6, PBUF=10, OBUF=4)


def build_nc(C, T, B):
    CT = C // 128          # contraction tiles (16)
    NCH = T // AC          # chunks per batch (4)
    BT = B * T
    KPB = T // 128         # key blocks per batch (16)

    nc = bacc.Bacc("TRN2", target_bir_lowering=False, debug=False)

    xTr = nc.dram_tensor("xTr", [128, CT, BT], BF16, kind="ExternalInput")
    wqr = nc.dram_tensor("wqr", [128, CT, 256], BF16, kind="ExternalInput")
    wkv = nc.dram_tensor("wkv", [128, CT, 128], BF16, kind="ExternalInput")
    idn = nc.dram_tensor("idn", [64, 64], BF16, kind="ExternalInput")
    wor = nc.dram_tensor("wor", [128, 2, C], BF16, kind="ExternalInput")
    rqc = nc.dram_tensor("rqc", [128, T], BF16, kind="ExternalInput")
    rqs = nc.dram_tensor("rqs", [128, T], BF16, kind="ExternalInput")
    rkc = nc.dram_tensor("rkc", [64, T], BF16, kind="ExternalInput")
    rks = nc.dram_tensor("rks", [64, T], BF16, kind="ExternalInput")
    tri2 = nc.dram_tensor("tri2", [128, 2, 128], BF16, kind="ExternalInput")
    id2 = nc.dram_tensor("id2", [128, 128], BF16, kind="ExternalInput")
    out = nc.dram_tensor("out", [BT, C], BF16, kind="ExternalOutput")

    with tile.TileContext(nc) as tc, ExitStack() as ctx:
        # PSUM: pj 2 + ss 4 + yy 2 = 8 banks
        pj = ctx.enter_context(tc.tile_pool(name="pj", bufs=2, space="PSUM"))
        ss = ctx.enter_context(tc.tile_pool(name="ss", bufs=2, space="PSUM"))
        yy = ctx.enter_context(tc.tile_pool(name="yy", bufs=2, space="PSUM"))

        cst = ctx.enter_context(tc.tile_pool(name="cst", bufs=1))
        xcp = ctx.enter_context(tc.tile_pool(name="xcp", bufs=3))
        ktp = ctx.enter_context(tc.tile_pool(name="ktp", bufs=2))
        vpp = ctx.enter_context(tc.tile_pool(name="vpp", bufs=2))
        qsp = ctx.enter_context(tc.tile_pool(name="qsp", bufs=4))
        qtp = ctx.enter_context(tc.tile_pool(name="qtp", bufs=8))
        tmp = ctx.enter_context(tc.tile_pool(name="tmp", bufs=4))
        ppp = ctx.enter_context(tc.tile_pool(name="ppp", bufs=PARAMS["PBUF"]))
        ytp = ctx.enter_context(tc.tile_pool(name="ytp", bufs=4))
        osp = ctx.enter_context(tc.tile_pool(name="osp", bufs=4))

        # ---- constants ----
        XW = cst.tile([128, CT, 256], BF16, tag="XW")
        KVW = cst.tile([128, CT, 128], BF16, tag="KVW")
        IDN = cst.tile([64, 64], BF16, tag="IDN")
        ID2 = cst.tile([128, 128], BF16, tag="ID2")
        OW = cst.tile([128, 2, C], BF16, tag="OW")
        RQC = cst.tile([128, T], BF16, tag="RQC")
        RQS = cst.tile([128, T], BF16, tag="RQS")
        RKC = cst.tile([64, T], BF16, tag="RKC")
        RKS = cst.tile([64, T], BF16, tag="RKS")
        TRI = cst.tile([128, 2, 128], BF16, tag="TRI")

        PROJ_ROPES = {}
        XC = {}   # (b, ch) -> x chunk tile [128, CT, AC]
        KT = {}   # b -> [64, T]
        VP = {}   # b -> [128, KPB, 65]
        QT = {}   # (ch%2, h) -> [64, AC]
        YN = {}   # b -> [128, KPB, 256] normalized Y in [t, head*d] layout
        YT = {}   # (b, cl) -> [128, T]

        def emit_xdma(b, ch):
            t = xcp.tile([128, CT, AC], BF16, tag="XC", name=f"XC{b}_{ch}")
            nc.sync.dma_start(t[:, :, :], xTr[:, :, b * T + ch * AC:b * T + (ch + 1) * AC])
            XC[(b, ch)] = t

        def fillers_proj(b, ch):
            """Closures projecting chunk (b, ch): K|V packed, Q0, Q1.

            Wk and Wv ride in one [128c, 128] stationary, so K^T and V^T come
            out of a single [128, AC] moving stream (half the PE rows of
            separate K/V passes). V^T is re-oriented to [t, d] with cheap PE
            transposes (64 out-rows each)."""
            xc = XC[(b, ch)]
            tcol = ch * AC
            res = []

            pkv_box = []
            ks_box, vs_box, qs_box = [], [], {}

            def kv_mm(c0):
                if c0 == 0:
                    pkv_box.append(pj.tile([128, AC], F32, tag="pj", name=f"PKV{b}_{ch}"))
                pkv = pkv_box[0]
                for c in range(c0, c0 + 4):
                    nc.tensor.matmul(pkv[:], KVW[:, c, :], xc[:, c, :],
                                     start=(c == 0), stop=(c == CT - 1))

            def kv_copy():
                ks = qsp.tile([64, AC], BF16, tag="KS", name=f"KS{b}_{ch}")
                nc.vector.tensor_copy(ks[:], pkv_box[0][0:64, :])
                ks_box.append(ks)
                vs = qsp.tile([64, AC], BF16, tag="VS", name=f"VS{b}_{ch}")
                nc.vector.tensor_copy(vs[:], pkv_box[0][64:128, :])
                vs_box.append(vs)

            def k_rope():
                ks = ks_box[0]
                kt = KT[b]
                t1 = tmp.tile([64, AC], BF16, tag="kt1")
                t2 = tmp.tile([64, AC], BF16, tag="kt2")
                nc.vector.tensor_mul(t1[:], ks[:], RKC[:, tcol:tcol + AC])
                nc.vector.tensor_mul(t2[0:32, :], ks[32:64, :], RKS[32:64, tcol:tcol + AC])
                nc.vector.tensor_mul(t2[32:64, :], ks[0:32, :], RKS[0:32, tcol:tcol + AC])
                nc.vector.tensor_add(kt[:, tcol:tcol + AC], t1[:], t2[:])

            def v_trans():
                pvt = pj.tile([128, 4, 64], BF16, tag="pj", name=f"PVT{b}_{ch}")
                for tb in range(4):
                    nc.tensor.transpose(pvt[:, tb, :],
                                        vs_box[0][:, tb * 128:(tb + 1) * 128],
                                        IDN[:, :])
                nc.vector.tensor_copy(VP[b][:, ch * 4:(ch + 1) * 4, 0:64],
                                      pvt[:, :, :])

            def q_mm(c0, hp, pq_box):
                if c0 == 0:
                    pq_box.append(pj.tile([128, AC], F32, tag="pj", name=f"PQ{b}_{ch}_{hp}"))
                pq = pq_box[0]
                for c in range(c0, c0 + 4):
                    nc.tensor.matmul(pq[:], XW[:, c, hp * 128:(hp + 1) * 128], xc[:, c, :],
                                     start=(c == 0), stop=(c == CT - 1))

            def q_copy(hp, pq_box):
                qs = qsp.tile([128, AC], BF16, tag="QS", name=f"QS{b}_{ch}_{hp}")
                nc.vector.tensor_copy(qs[:], pq_box[0][:])
                qs_box[hp] = qs

            def q_rope(hp):
                qs = qs_box[hp]
                t1 = tmp.tile([128, AC], BF16, tag="qt1")
                t2 = tmp.tile([128, AC], BF16, tag="qt2")
                nc.vector.tensor_mul(t1[:], qs[:], RQC[:, tcol:tcol + AC])
                for b0 in (0, 64):
                    nc.vector.tensor_mul(t2[b0:b0 + 32, :], qs[b0 + 32:b0 + 64, :],
                                         RQS[b0 + 32:b0 + 64, tcol:tcol + AC])
                    nc.vector.tensor_mul(t2[b0 + 32:b0 + 64, :], qs[b0:b0 + 32, :],
                                         RQS[b0:b0 + 32, tcol:tcol + AC])
                for hl in range(2):
                    h = 2 * hp + hl
                    qt = qtp.tile([64, AC], BF16, tag="QT", name=f"QT{b}_{ch}_{h}")
                    nc.vector.tensor_add(qt[:], t1[hl * 64:hl * 64 + 64, :],
                                         t2[hl * 64:hl * 64 + 64, :])
                    QT[(ch % 2, h)] = qt

            # psum->sbuf copies right behind each chain (frees pj bufs fast);
            # ropes returned separately so chunk+2 pushes can defer them
            # (QT parity), but the chunk+1 push weaves them inline
            for c0 in range(0, CT, 4):
                res.append((875, lambda c0=c0: kv_mm(c0)))
            res.append((15, kv_copy))
            pq_boxes = [[], []]
            for hp in range(2):
                for c0 in range(0, CT, 4):
                    res.append((875, lambda c0=c0, hp=hp: q_mm(c0, hp, pq_boxes[hp])))
                res.append((10, lambda hp=hp: q_copy(hp, pq_boxes[hp])))
                if hp == 0:
                    res.append((160, v_trans))
            ropes = [(150, k_rope), (150, lambda: q_rope(0)),
                     (150, lambda: q_rope(1))]
            return res, ropes

        def weave(ca, ra):
            """Interleave ropes right behind their producing copies:
            k_rope after kv_copy (idx 4), q_rope0 after q_copy0 (idx 9),
            q_rope1 after q_copy1 (idx 16)."""
            return (ca[0:5] + [ra[0]] + ca[5:10] + [ra[1]]
                    + ca[10:16] + [ra[2]] + ca[16:])

        TAIL = [False]

        def fillers_outproj(b, ch):
            """Closures for out projection of chunk (b, ch) (needs YT cols).

            Two co-columns per group share one [128,1024] staging tile and a
            single DMA — halves the serialized HWDGE holds (625ns each)."""
            res = []

            os_box = {}
            did0 = {}

            def po_group(tt, co2):
                trow = b * T + ch * AC + tt * 128
                if co2 == 0:
                    os_box[tt] = osp.tile([128, 2048], BF16, tag="OS", name=f"OS{b}_{ch}_{tt}", bufs=PARAMS["OBUF"])
                os_ = os_box[tt]
                for j in range(2):
                    co = 2 * co2 + j
                    po = pj.tile([128, 512], F32, tag="pj", name=f"PO{b}_{ch}_{tt}_{co}")
                    for cl in range(2):
                        nc.tensor.matmul(po[:], YT[(b, cl)][:, ch * 4 * 128 + tt * 128:ch * 4 * 128 + (tt + 1) * 128],
                                         OW[:, cl, co * 512:(co + 1) * 512],
                                         start=(cl == 0), stop=(cl == 1))
                    if j == 0:
                        nc.vector.tensor_copy(os_[:, co2 * 1024:co2 * 1024 + 512], po[:])
                    else:
                        nc.vector.tensor_copy(os_[:, co2 * 1024 + 512:co2 * 1024 + 1024], po[:])
                if co2 == 0:
                    did0[tt] = TAIL[0]
                    if TAIL[0]:
                        nc.sync.dma_start(out[trow:trow + 128, 0:1024], os_[:, 0:1024])
                elif did0[tt]:
                    nc.sync.dma_start(out[trow:trow + 128, 1024:2048], os_[:, 1024:2048])
                else:
                    nc.sync.dma_start(out[trow:trow + 128, :], os_[:])

            for tt in range(4):
                for co2 in range(C // 1024):
                    res.append((880, lambda tt=tt, co2=co2: po_group(tt, co2)))
            return res

        projq = deque()   # (cost_ns, closure) — must drain before next chunk
        sideq = deque()   # (cost, closure, min_epoch) — out-proj, drain lazily
        qcost = [0]       # total cost queued
        pcost = [0]       # proj cost queued
        epoch = [0]       # attention blocks emitted so far
        PROJ_FLOOR = PARAMS["FLOOR"]  # proj work kept for the chunk boundary

        def push(q, items):
            q.extend(items)
            qcost[0] += sum(it[0] for it in items)
            if q is projq:
                pcost[0] += sum(it[0] for it in items)

        reserve = [0]

        def side_ready():
            return (sideq and qcost[0] > reserve[0]
                    and epoch[0] >= sideq[0][2])

        def head_cost():
            if projq:
                return projq[0][0]
            if side_ready():
                return sideq[0][0]
            return None

        def fill(budget):
            while budget > 0:
                if projq and pcost[0] > PROJ_FLOOR:
                    c, f = projq.popleft()
                    pcost[0] -= c
                elif side_ready():
                    c, f, _ = sideq.popleft()
                elif projq:
                    c, f = projq.popleft()
                    pcost[0] -= c
                else:
                    return
                f()
                qcost[0] -= c
                budget -= c

        def flush_proj():
            while projq:
                c, f = projq.popleft()
                qcost[0] -= c
                pcost[0] -= c
                f()

        def flush_all():
            flush_proj()
            while sideq:
                c, f, _ = sideq.popleft()
                qcost[0] -= c
                f()

        def normalize_flip(b, ch, yj, pr, last=False):
            """Stage Y [t, 4qb, d|den] to SBUF f32; den is per-PARTITION in
            this orientation, so one recip + per-partition-scalar muls —
            no broadcast needed."""
            for j in range(2):
                ys = tmp.tile([128, 4, 65], F32, tag="YS", name=f"YS{b}_{ch}_{pr}_{j}")
                nc.vector.tensor_copy(ys[:, :, :], yj[j][:, :, 0:65])
                rd = tmp.tile([128, 4], F32, tag="RD", name=f"RD{b}_{ch}_{pr}_{j}")
                nc.vector.reciprocal(rd[:, :], ys[:, :, 64])
                h = 2 * pr + j
                for qb in range(4):
                    nc.vector.tensor_scalar_mul(
                        YN[b][:, ch * 4 + qb, h * 64:(h + 1) * 64],
                        ys[:, qb, 0:64], rd[:, qb:qb + 1])

        def fillers_ytrans(b, ch, cl):
            """Transpose normalized Y [t, hd] back to [hd, t] for out-proj."""
            def yt_go():
                pvy = pj.tile([128, 4, 128], BF16, tag="pj", name=f"PYT{b}_{ch}_{cl}")
                for tt in range(4):
                    nc.tensor.transpose(pvy[:, tt, :],
                                        YN[b][:, ch * 4 + tt, cl * 128:(cl + 1) * 128],
                                        ID2[:, :])
                nc.vector.tensor_copy(YT[(b, cl)][:, ch * AC:(ch + 1) * AC],
                                      pvy[:, :, :])
            return [(280, yt_go)]

        def attention_chunk(b, ch, budget, last_chunk):
            """Both head-pair passes of chunk (b, ch) as one pipelined stream.

            AV is q-stationary: each exp'd P block [keys, 128q] loads as the
            (free) stationary and V [keys, 65] streams through, so an entry
            costs 65-row matmuls instead of 512 — Y lands as [q, d|den]."""
            kt = KT[b]
            vp = VP[b]
            kis = [("d", l) for l in range(4)] + [("f", k) for k in range(ch * 4)]
            n = len(kis)
            yps = {}
            pend = deque()

            def emit_av(e):
                p2, f0, pr, idx, ki_ = e
                qb0 = f0 // 128
                for j in range(2):
                    for qb in range(qb0, 4):
                        # ONE accumulation group per head-bank: start zeroes
                        # the whole 2kb zero region, so only the first matmul
                        # into the bank starts and only the last stops.
                        nc.tensor.matmul(yps[pr][j][:, qb, 0:65],
                                         p2[:, j, qb * 128:(qb + 1) * 128],
                                         vp[:, ki_, :],
                                         start=(idx == 0 and qb == 0),
                                         stop=(idx == n - 1 and qb == 3),
                                         skip_group_check=True)
                if idx == n - 1:
                    normalize_flip(b, ch, yps[pr], pr, last_chunk and pr == 1)
                    push(sideq, [(c, f, epoch[0] + PARAMS["YGATE"])
                                 for c, f in fillers_ytrans(b, ch, pr)])

            for pr in range(2):
                for i, (kind, v) in enumerate(kis):
                    ki = ch * 4 + v if kind == "d" else v
                    f0 = v * 128 if kind == "d" else 0
                    if i == 0:
                        yps[pr] = [yy.tile([128, 4, 128], F32, tag="yy",
                                           name=f"Y{b}_{ch}_{pr}_{j}")
                                   for j in range(2)]
                    s2 = ss.tile([128, 2, 512], F32, tag="ss", name=f"S{b}_{ch}_{pr}_{i}")
                    for j in range(2):
                        h = 2 * pr + j
                        nc.tensor.matmul(s2[:, j, f0:512], kt[:, ki * 128:(ki + 1) * 128],
                                         QT[(ch % 2, h)][:, f0:512], start=True, stop=True)
                    p2 = ppp.tile([128, 2, 512], BF16, tag="P2")
                    nc.scalar.activation(p2[:, :, f0:512], s2[:, :, f0:512], EXP)
                    if kind == "d":
                        nc.gpsimd.tensor_mul(p2[:, :, f0:f0 + 128], p2[:, :, f0:f0 + 128], TRI[:, :, :])
                    pend.append((p2, f0, pr, i, ki))
                    epoch[0] += 1
                    if len(pend) > PARAMS["PEND"]:
                        emit_av(pend.popleft())
                    fill(PARAMS["FILL"])
            while pend:
                emit_av(pend.popleft())

        # ---- PE warmup: ramp the p-state while the first DMAs land.
        #      8 matmuls end ~3.2us — exactly when the ramp completes and
        #      the first weights + x rows have arrived. ----
        WRM = cst.tile([128, 512], BF16, tag="WRM")
        nc.vector.memset(WRM[:, :], 0.0)
        pwarm = pj.tile([128, 512], F32, tag="pj", name="PWARM")
        for _ in range(PARAMS["WPRE"]):
            nc.tensor.matmul(pwarm[:], WRM[:, 0:128], WRM[:], start=True, stop=True)

        # ---- preamble DMAs (need-ordered; x chunk 0 and XW split so each
        #      proj chain can start as soon as its first operands arrive) ----
        x00 = xcp.tile([128, CT, AC], BF16, tag="XC", name="XC0_0")
        XC[(0, 0)] = x00
        nc.sync.dma_start(KVW[:, 0:8, :], wkv[:, 0:8, :])
        nc.sync.dma_start(x00[:, 0:4, :], xTr[:, 0:4, 0:AC])
        nc.sync.dma_start(KVW[:, 8:CT, :], wkv[:, 8:CT, :])
        for q in range(1, 4):
            nc.sync.dma_start(x00[:, 4 * q:4 * (q + 1), :],
                              xTr[:, 4 * q:4 * (q + 1), 0:AC])
        nc.sync.dma_start(IDN[:], idn[:])
        nc.sync.dma_start(RKC[:], rkc[:])
        nc.sync.dma_start(RKS[:], rks[:])
        nc.sync.dma_start(XW[:, 0:4, :], wqr[:, 0:4, :])
        nc.sync.dma_start(XW[:, 4:CT, :], wqr[:, 4:CT, :])
        nc.sync.dma_start(RQC[:], rqc[:])
        nc.sync.dma_start(RQS[:], rqs[:])
        nc.sync.dma_start(TRI[:, :, :], tri2[:, :, :])
        nc.sync.dma_start(ID2[:, :], id2[:, :])
        nc.sync.dma_start(OW[:, :, :], wor[:, :, :])
        emit_xdma(0, 1)

        for b in range(B):
            KT[b] = ktp.tile([64, T], BF16, tag="KT", name=f"KT{b}")
            VP[b] = vpp.tile([128, KPB, 65], BF16, tag="VP", name=f"VP{b}")
            nc.vector.memset(VP[b][:, :, 64:65], 1.0)
            YN[b] = ytp.tile([128, KPB, 256], BF16, tag="YN", name=f"YN{b}",
                             bufs=2)
            for cl in range(2):
                YT[(b, cl)] = ytp.tile([128, T], BF16, tag="YT", name=f"YT{b}_{cl}")

        def warm():
            nc.tensor.matmul(pwarm[:], WRM[:, 0:128], WRM[:], start=True, stop=True)

        ca, ra = fillers_proj(0, 0)
        for _, c in ca + ra:
            c()
        for _ in range(PARAMS["WPOST"]):
            warm()

        def succ(b, ch, k):
            t = b * NCH + ch + k
            return (t // NCH, t % NCH) if t < B * NCH else None

        pushed = set()
        for b in range(B):
            for ch in range(NCH):
                flush_proj()  # this chunk's proj must be complete
                nxt = succ(b, ch, 2)
                if nxt:
                    emit_xdma(*nxt)
                # queue fillers: chains for chunk+1 (if new) + its ropes,
                # then chains of chunk+2 (QT-parity-safe extra supply)
                nxt = succ(b, ch, 1)
                if nxt:
                    if nxt not in pushed:
                        ca, ra = fillers_proj(*nxt)
                        push(projq, ca)
                        pushed.add(nxt)
                        PROJ_ROPES[nxt] = ra
                    push(projq, PROJ_ROPES.pop(nxt))
                nxt = succ(b, ch, 2)
                if ch >= 2 and nxt and nxt not in pushed:
                    ca, ra = fillers_proj(*nxt)
                    push(projq, ca)
                    pushed.add(nxt)
                    PROJ_ROPES[nxt] = ra
                last = b == B - 1 and ch == NCH - 1
                reserve[0] = PARAMS["RESERVE"] if last else 0
                attention_chunk(b, ch, 800, last)
                # PO of this chunk only becomes eligible a few attention
                # blocks into the NEXT chunk, so normalize has drained.
                push(sideq, [(c, f, epoch[0] + PARAMS["EGATE"])
                             for c, f in fillers_outproj(b, ch)])
        TAIL[0] = True
        flush_all()

    nc.compile()
    return nc


def rope_tables(T, scale):
    inv = 1.0 / (ROPE_BASE ** (np.arange(0, D, 2, dtype=np.float32) / D))
    t = np.arange(T, dtype=np.float32)
    freqs = np.outer(t, inv)
    emb = np.concatenate([freqs, freqs], -1)
    cos = np.cos(emb).T.astype(np.float32) * scale
    sin = np.sin(emb).T.astype(np.float32) * scale
    sinX = np.empty((64, T), np.float32)
    sinX[0:32] = sin[32:64]
    sinX[32:64] = -sin[0:32]
    return np.ascontiguousarray(cos), np.ascontiguousarray(sinX)


def _pk(a, nblk):
    """[nblk*128, F] -> [128, nblk, F] contiguous bf16."""
    n, f = a.shape
    return np.ascontiguousarray(
        a.reshape(nblk, 128, f).transpose(1, 0, 2)).astype(BF16_NP)


def make_inputs(x, Wq, Wk, Wv, Wo):
    B, T, C = x.shape
    CT = C // 128
    xT = np.ascontiguousarray(x.reshape(B * T, C).T)
    qc, qs = rope_tables(T, 1.0 / np.sqrt(D).astype(np.float32))
    kc, ks = rope_tables(T, 1.0)
    tri = np.triu(np.ones((128, 128), np.float32))
    common = {
        "xTr": _pk(xT, CT),
        "rqc": np.concatenate([qc, qc], 0).astype(BF16_NP),
        "rqs": np.concatenate([qs, qs], 0).astype(BF16_NP),
        "rkc": kc.astype(BF16_NP),
        "rks": ks.astype(BF16_NP),
        "tri2": np.ascontiguousarray(
            np.stack([tri, tri], 1)).astype(BF16_NP),
        "idn": np.eye(64, dtype=np.float32).astype(BF16_NP),
        "id2": np.eye(128, dtype=np.float32).astype(BF16_NP),
    }
    in_maps = []
    for i in range(8):
        m = dict(common)
        m["wqr"] = _pk(np.ascontiguousarray(Wq[:, i * 256:(i + 1) * 256]), CT)
        m["wkv"] = _pk(np.ascontiguousarray(np.concatenate(
            [Wk[:, i * 64:(i + 1) * 64], Wv[:, i * 64:(i + 1) * 64]], 1)), CT)
        m["wor"] = _pk(np.ascontiguousarray(Wo[i * 256:(i + 1) * 256, :]), 2)
        in_maps.append(m)
    return in_maps


_NC_CACHE = {}


def _get_nc(C, T, B):
    key = (C, T, B)
    if key not in _NC_CACHE:
        _NC_CACHE[key] = build_nc(C, T, B)
    return _NC_CACHE[key]


def run(x, Wq, Wk, Wv, Wo, trace=False):
    from concourse.bass_utils import run_bass_kernel_spmd

    B, T, C = x.shape
    nc = _get_nc(C, T, B)
    in_maps = make_inputs(x, Wq, Wk, Wv, Wo)
    for attempt in range(3):
        try:
            res = run_bass_kernel_spmd(nc, in_maps, list(range(8)), trace=trace)
        except (ImportError, ModuleNotFoundError):
            res = run_bass_kernel_spmd(nc, in_maps, list(range(8)), trace=False)
        acc = res.results[0]["out"].astype(np.float32)
        for i in range(1, 8):
            acc = acc + res.results[i]["out"].astype(np.float32)
        if np.isfinite(acc).all():
            break
    return acc.reshape(B, T, C), res


def kernel(x, Wq, Wk, Wv, Wo):
    out, _ = run(x, Wq, Wk, Wv, Wo, trace=False)
    return out

